# revision 7
# baseline (speedup 1.0000x reference)
"""Trainium2 Bass kernel for JEPA world-model loss (topk masking + MLP + masked L1).

Strategy: data-parallel over batch across 8 cores. All device tensors are in
feature-major ("transposed") layout so both matmuls consume weights in their
natural DRAM layout with zero on-device transposes:
  mm1: hT[hid,b] = W1T[d,hid].T @ xmT[d,b]   (W1T = W1.T, host-prepped)
  mm2: pT[d,b]   = W2T[hid,d].T @ hT[hid,b]  (W2T = W2.T, host-prepped)
The per-row top-k (k=1024 smallest of 4096 uniform noise values) threshold is
found exactly by 24-step integer bisection on the 2^-24 grid (jax uniform
values are multiples of 2^-23), using fused compare+reduce tensor_scalar ops
on rows-in-partitions noise tiles. Loss partials are reduced per (d-chunk,
b-half) tile via activation(Abs, accum_out) and summed on host.
All phases run per b-half (512 rows) so SBUF holds only one half's hidden
activations; W1/W2 stream once per half.
"""

import sys

sys.path.insert(0, "/opt/trn_rl_repo")

import numpy as np
import ml_dtypes

import concourse.bass as bass
import concourse.mybir as mybir
import concourse.tile as tile
from concourse import bacc
from concourse.bass_utils import run_bass_kernel_spmd

B, D, HID = 8192, 4096, 8192
NCORES = 8
BS = B // NCORES          # 1024 rows per core
NMASK = D // 4            # 1024 masked per row
GRID_BITS = 24            # bisection grid 2^-24 (superset of jax's 2^-23 grid)
RT = BS // 128            # 8 row-tiles per core
DC = D // 128             # 32 d chunks
HC = HID // 128           # 64 hid chunks
HALF = BS // 2            # 512
F32 = mybir.dt.float32
BF16 = mybir.dt.bfloat16
ALU = mybir.AluOpType
AF = mybir.ActivationFunctionType

_cache = {}


def _build():
    nc = bacc.Bacc("TRN2")
    xT = nc.dram_tensor("xT", [D, BS], BF16, kind="ExternalInput")
    nzN = nc.dram_tensor("nzN", [BS, D], F32, kind="ExternalInput")
    nzT = nc.dram_tensor("nzT", [D, BS], F32, kind="ExternalInput")
    tT = nc.dram_tensor("tT", [D, BS], BF16, kind="ExternalInput")
    w1t = nc.dram_tensor("w1t", [D, HID], BF16, kind="ExternalInput")
    w2t = nc.dram_tensor("w2t", [HID, D], BF16, kind="ExternalInput")
    b1pc = nc.dram_tensor("b1pc", [128, HC], F32, kind="ExternalInput")
    b2pc = nc.dram_tensor("b2pc", [128, DC], F32, kind="ExternalInput")
    partials = nc.dram_tensor("partials", [128, 2 * DC], F32, kind="ExternalOutput")
    vthr_d = nc.dram_tensor("vthr_scratch", [BS, 1], F32)

    with tile.TileContext(nc) as tc:
        with (
            tc.tile_pool(name="noise", bufs=2) as noise_pool,
            tc.tile_pool(name="cnt_scratch", bufs=2) as cs_pool,
            tc.tile_pool(name="small", bufs=2) as small_pool,
            tc.tile_pool(name="bcast", bufs=2) as bc_pool,
            tc.tile_pool(name="xm", bufs=DC) as xm_pool,
            tc.tile_pool(name="ht", bufs=HC) as ht_pool,
            tc.tile_pool(name="wstream", bufs=4) as w_pool,
            tc.tile_pool(name="io_small", bufs=1) as io_pool,
            tc.tile_pool(name="app", bufs=3) as app_pool,
            tc.tile_pool(name="loss", bufs=2) as loss_pool,
            tc.tile_pool(name="psum1", bufs=4, space="PSUM") as ps1,
            tc.tile_pool(name="psum2", bufs=4, space="PSUM") as ps2,
        ):
            b1_sb = io_pool.tile([128, HC], F32, tag="b1")
            nc.sync.dma_start(out=b1_sb[:], in_=b1pc[:])
            b2_sb = io_pool.tile([128, DC], F32, tag="b2")
            nc.sync.dma_start(out=b2_sb[:], in_=b2pc[:])
            parts_sb = io_pool.tile([128, 2 * DC], F32, tag="parts")

            for h in range(2):
                # ---- Phase A: bisection for this half's 4 row-tiles ----
                for rt4 in range(RT // 2):
                    rt = h * (RT // 2) + rt4
                    nz = noise_pool.tile([128, D], F32, tag="nz")
                    nc.sync.dma_start(
                        out=nz[:], in_=nzN[rt * 128 : (rt + 1) * 128, :]
                    )
                    acc = small_pool.tile([128, 1], F32, tag=f"acc{rt}")
                    nc.vector.memset(acc[:], 0.0)
                    scratch = cs_pool.tile([128, D], BF16, tag="cs")
                    for p in range(GRID_BITS - 1, -1, -1):
                        thr = small_pool.tile([128, 1], F32, tag=f"thr{rt}")
                        nc.vector.tensor_scalar(
                            out=thr[:], in0=acc[:],
                            scalar1=float(2**p) - 0.5,
                            scalar2=float(2.0**-GRID_BITS),
                            op0=ALU.add, op1=ALU.mult,
                        )
                        cnt = small_pool.tile([128, 1], F32, tag=f"cnt{rt}")
                        nc.vector.tensor_scalar(
                            out=scratch[:], in0=nz[:],
                            scalar1=thr[:], scalar2=None,
                            op0=ALU.is_le, op1=ALU.add,
                            accum_out=cnt[:],
                        )
                        selp = small_pool.tile([128, 1], F32, tag=f"selp{rt}")
                        nc.vector.tensor_scalar(
                            out=selp[:], in0=cnt[:],
                            scalar1=float(NMASK), scalar2=float(2**p),
                            op0=ALU.is_lt, op1=ALU.mult,
                        )
                        nc.vector.tensor_add(out=acc[:], in0=acc[:], in1=selp[:])
                    vthr = small_pool.tile([128, 1], F32, tag=f"vthr{rt}")
                    nc.vector.tensor_scalar(
                        out=vthr[:], in0=acc[:],
                        scalar1=0.5, scalar2=float(2.0**-GRID_BITS),
                        op0=ALU.add, op1=ALU.mult,
                    )
                    nc.sync.dma_start(
                        out=vthr_d[rt * 128 : (rt + 1) * 128, :], in_=vthr[:]
                    )

                # ---- Phase B: broadcast thresholds along partitions ----
                bct = bc_pool.tile([128, HALF], F32, tag="vbc")
                nc.sync.dma_start(
                    out=bct[:],
                    in_=bass.AP(vthr_d, h * HALF, [[0, 128], [1, HALF]]),
                )

                # ---- Phase C: apply mask in transposed layout ----
                xm = []
                for c in range(DC):
                    nt = app_pool.tile([128, HALF], F32, tag="nzt_app")
                    nc.sync.dma_start(
                        out=nt[:],
                        in_=nzT[c * 128 : (c + 1) * 128, h * HALF : (h + 1) * HALF],
                    )
                    xt = app_pool.tile([128, HALF], BF16, tag="xt_app")
                    nc.sync.dma_start(
                        out=xt[:],
                        in_=xT[c * 128 : (c + 1) * 128, h * HALF : (h + 1) * HALF],
                    )
                    keep = app_pool.tile([128, HALF], BF16, tag="keep")
                    nc.vector.tensor_tensor(
                        out=keep[:], in0=nt[:], in1=bct[:], op=ALU.is_gt
                    )
                    xm_t = xm_pool.tile([128, HALF], BF16, tag="xm", name=f"xm_{h}_{c}")
                    nc.vector.tensor_tensor(
                        out=xm_t[:], in0=xt[:], in1=keep[:], op=ALU.mult
                    )
                    xm.append(xm_t)

                # ---- Phase D: mm1 + gelu for this half ----
                ht = []
                for hcg in range(HC // 2):
                    ps = [
                        ps1.tile([128, HALF], F32, tag="mm1", name=f"ps_{h}_{hcg}_{i}")
                        for i in range(2)
                    ]
                    for c in range(DC):
                        wt = w_pool.tile([128, 256], BF16, tag="w1g")
                        nc.sync.dma_start(
                            out=wt[:],
                            in_=w1t[c * 128 : (c + 1) * 128,
                                    hcg * 256 : (hcg + 1) * 256],
                        )
                        for sub in range(2):
                            nc.tensor.matmul(
                                out=ps[sub][:],
                                lhsT=wt[:, sub * 128 : (sub + 1) * 128],
                                rhs=xm[c][:],
                                start=(c == 0), stop=(c == DC - 1),
                            )
                    for sub in range(2):
                        hc = hcg * 2 + sub
                        ht_t = ht_pool.tile([128, HALF], BF16, tag="ht", name=f"ht_{h}_{hc}")
                        nc.scalar.activation(
                            out=ht_t[:], in_=ps[sub][:],
                            func=AF.Gelu, bias=b1_sb[:, hc : hc + 1],
                        )
                        ht.append(ht_t)

                # ---- Phase E: mm2 + masked-abs-diff loss for this half ----
                for cg in range(DC // 2):
                    pd = [
                        ps2.tile([128, HALF], F32, tag="mm2", name=f"pd_{h}_{cg}_{i}")
                        for i in range(2)
                    ]
                    for hc in range(HC):
                        wt2 = w_pool.tile([128, 256], BF16, tag="w2g")
                        nc.sync.dma_start(
                            out=wt2[:],
                            in_=w2t[hc * 128 : (hc + 1) * 128,
                                    cg * 256 : (cg + 1) * 256],
                        )
                        for sub in range(2):
                            nc.tensor.matmul(
                                out=pd[sub][:],
                                lhsT=wt2[:, sub * 128 : (sub + 1) * 128],
                                rhs=ht[hc][:],
                                start=(hc == 0), stop=(hc == HC - 1),
                            )
                    for sub in range(2):
                        c = cg * 2 + sub
                        ttile = loss_pool.tile([128, HALF], BF16, tag="tT_l")
                        nc.sync.dma_start(
                            out=ttile[:],
                            in_=tT[c * 128 : (c + 1) * 128,
                                   h * HALF : (h + 1) * HALF],
                        )
                        nt2 = loss_pool.tile([128, HALF], F32, tag="nzt_l")
                        nc.sync.dma_start(
                            out=nt2[:],
                            in_=nzT[c * 128 : (c + 1) * 128,
                                    h * HALF : (h + 1) * HALF],
                        )
                        cmp = loss_pool.tile([128, HALF], BF16, tag="cmp_l")
                        nc.vector.tensor_tensor(
                            out=cmp[:], in0=nt2[:], in1=bct[:], op=ALU.is_le
                        )
                        # diff = (pred + b2) - teacher, via two-scalar ts + tt
                        pb = loss_pool.tile([128, HALF], F32, tag="pb_l")
                        nc.vector.tensor_scalar(
                            out=pb[:], in0=pd[sub][:],
                            scalar1=b2_sb[:, c : c + 1], scalar2=None,
                            op0=ALU.add,
                        )
                        diff = loss_pool.tile([128, HALF], F32, tag="diff_l")
                        nc.vector.tensor_tensor(
                            out=diff[:], in0=pb[:], in1=ttile[:], op=ALU.subtract
                        )
                        w = loss_pool.tile([128, HALF], F32, tag="w_l")
                        nc.vector.tensor_tensor(
                            out=w[:], in0=diff[:], in1=cmp[:], op=ALU.mult
                        )
                        junk = loss_pool.tile([128, HALF], BF16, tag="junk_l")
                        nc.scalar.activation(
                            out=junk[:], in_=w[:], func=AF.Abs,
                            accum_out=parts_sb[:, c * 2 + h : c * 2 + h + 1],
                        )

            nc.sync.dma_start(out=partials[:], in_=parts_sb[:])

    nc.compile()
    return nc


def kernel(student_latent, teacher_latent, mask_noise, W1, b1, W2, b2):
    if "nc" not in _cache:
        _cache["nc"] = _build()
    nc = _cache["nc"]

    bf = ml_dtypes.bfloat16
    w1t = np.ascontiguousarray(W1.T).astype(bf)
    w2t = np.ascontiguousarray(W2.T).astype(bf)
    b1pc = np.ascontiguousarray(b1.reshape(HC, 128).T).astype(np.float32)
    b2pc = np.ascontiguousarray(b2.reshape(DC, 128).T).astype(np.float32)

    in_maps = []
    for i in range(NCORES):
        sl = slice(i * BS, (i + 1) * BS)
        xs = student_latent[sl]
        ns = mask_noise[sl]
        ts = teacher_latent[sl]
        in_maps.append({
            "xT": np.ascontiguousarray(xs.T).astype(bf),
            "nzN": np.ascontiguousarray(ns).astype(np.float32),
            "nzT": np.ascontiguousarray(ns.T).astype(np.float32),
            "tT": np.ascontiguousarray(ts.T).astype(bf),
            "w1t": w1t,
            "w2t": w2t,
            "b1pc": b1pc,
            "b2pc": b2pc,
        })

    _cache["last_in_maps"] = in_maps
    global _last_in_maps
    _last_in_maps = in_maps
    res = run_bass_kernel_spmd(nc, in_maps, core_ids=list(range(NCORES)))
    total = np.float64(0.0)
    for i in range(NCORES):
        total += np.asarray(res.results[i]["partials"], dtype=np.float64).sum()
    loss = np.float32(total / float(B * NMASK))
    return np.array(loss, dtype=np.float32)
